# revision 39
# baseline (speedup 1.0000x reference)
"""Trainium2 Bass kernel for nn_DGM_77318001263213 (dense_transformer).

Reference computation (per batch b of 16):
  dir_map = conv3x3_SAME(x, dc_w) + dc_b            [12, 64, 64]
  q = conv2x2_s2(dir_map, q_w) + q_b  -> [48, 1024]
  k = conv2x2_s2(dir_map, k_w) + k_b  -> [48, 1024]
  v = conv2x2_s2(x, v_w) + v_b        -> [1024, 1024]
  attn = softmax(q^T k, axis=-1)                    [1024, 1024]
  out[c, m] = sum_n v[c, n] * attn[m, n]            [1024, 1024]

Device mapping (data-parallel, 2 batches per core on 8 cores):
  * q,k are computed as ONE composite 4x4 stride-2 convolution of x
    (the 3x3 dc conv and 2x2 proj convs are both linear, so they fold
    into a single 4x4 kernel on the host) with 96 output channels
    (q stacked with k) -- keeps PE matmul M=96 instead of M=12.
  * v conv computed transposed: V^T[n, oc] tiles via matmuls with
    lhsT = space-to-depth x (derived on device from the parity planes),
    rhs = reshaped v weights.
  * scores computed transposed: T[n, m] = S[m, n] = sum_c k[c,n] q[c,m],
    then E = exp(T) (no max subtraction; |S| <= ~25 so exp is safe),
    unnormalized U^T[m, c] = sum_n E[n, m] V^T[n, c], row sums
    D[m] = sum_n E[n, m] via ones-matmul, and out^T[m, c] =
    U^T[m, c] / D[m] applied as a per-partition scale on eviction.
  * matmul operands are bf16 (x, weights, E, V^T) except the scores
    matmul (QK/Qs stay fp32r: q/k noise feeds exp and is the one place
    extra mantissa bits pay for themselves).  PE rate is identical for
    bf16/fp32r (~250 ns per 512-col matmul, ~2.05 GHz sustained; fp8
    DoubleRow measured only 2x per instruction, which precision-mandated
    hi/lo splits would exactly cancel -- so no fp8).  bf16 halves DMA
    and SBUF instead.
  * host transposes out^T -> out at gather time.
"""
import os
import sys
import types
import numpy as np
from contextlib import ExitStack

for _p in ("/opt/trn_rl_repo", "/root/.axon_site/_ro/trn_rl_repo"):
    if os.path.isdir(_p) and _p not in sys.path:
        sys.path.insert(0, _p)

import ml_dtypes
import concourse.bacc as bacc
import concourse.bass as bass
import concourse.tile as tile
import concourse.mybir as mybir
from concourse import bass_utils

F32 = mybir.dt.float32
F32R = mybir.dt.float32r
BF16 = mybir.dt.bfloat16
NP_BF16 = ml_dtypes.bfloat16
ts = bass.ts

NCORES = 8
BPC = 2          # batches per core
C_IN = 256
NPOS = 1024      # 32*32 output positions
OC = 1024        # v output channels


def _install_ntff_hook_shim():
    """Register the axon NTFF profile hook if the image's antenv lacks it.

    Only needed when BASS_TRACE=1; harmless otherwise."""
    if "antenv.axon_hooks" in sys.modules:
        return
    try:
        from trn_agent_boot.trn_boot import _ntff_profile_via_ctypes
        hook = _ntff_profile_via_ctypes("/opt/axon/libaxon_pjrt.so")
    except Exception:
        hook = None
    m = types.ModuleType("antenv.axon_hooks")
    m.get_axon_ntff_profile_hook = lambda: hook
    m.set_axon_ntff_profile_hook = lambda h: None
    sys.modules["antenv.axon_hooks"] = m


def build_program():
    """Build the per-core Bacc program (same program on all 8 cores)."""
    nc = bacc.Bacc(trn_type="TRN2", target_bir_lowering=False, debug=False)

    # padded x as 4 stride-2 parity planes, each duplicated at 2 column
    # shifts: xq[b, c, a*2+p, sv, r, j] = x_pad[b, c, 2r+a, 2(sv+j)+p].
    # The shift copies make every conv tap view a fully CONTIGUOUS
    # [128, 512] / [128, 1024] run (32-wide 64B-aligned rows), which the
    # PE ifmap fetcher streams at full rate (strided 33-wide views
    # measured ~263 ns vs 225 ns per matmul), and lets the v conv use
    # the planes directly as its stationary operand (no space-to-depth
    # copies on device at all).  Costs 2x the x DMA bytes -- DMA has
    # ~3x headroom over the PE here.
    # plane-major so each (half, plane) DMA reads one contiguous 540KB
    # region (strided per-channel reads measured ~150 GB/s vs wire 358)
    xq = nc.dram_tensor("xq", [BPC, 4, C_IN, 2, 33, 32], BF16, kind="ExternalInput")
    # wqk chunk order = (h, pl, u, v) == the exact consumption order of the
    # q/k conv loop, so the per-group DMAs stream just ahead of the matmuls
    wqk = nc.dram_tensor("wqk", [128, 32, 96], BF16, kind="ExternalInput")
    wv = nc.dram_tensor("wv", [128, 8, 1024], BF16, kind="ExternalInput")
    bqk = nc.dram_tensor("bqk", [96, 1], F32, kind="ExternalInput")
    bvr = nc.dram_tensor("bvr", [128, 1024], F32, kind="ExternalInput")
    # bf16 + tile-major output: each [128, 512] eviction writes one
    # contiguous 131KB region (strided writes measured ~37 GB/s -- they
    # were the entire end-of-program drain); host reassembles + converts
    o = nc.dram_tensor("o", [BPC, 8, 128, 1024], BF16, kind="ExternalOutput")

    EXP = mybir.ActivationFunctionType.Exp
    COPY = mybir.ActivationFunctionType.Copy

    with tile.TileContext(nc) as tc, ExitStack() as ctx:
        const = ctx.enter_context(tc.tile_pool(name="const", bufs=1))
        xpool = ctx.enter_context(tc.tile_pool(name="xpool", bufs=16))
        qkp = ctx.enter_context(tc.tile_pool(name="qkp", bufs=1))
        epool = ctx.enter_context(tc.tile_pool(name="epool", bufs=1))
        vtpool = ctx.enter_context(tc.tile_pool(name="vtpool", bufs=1))
        outp = ctx.enter_context(tc.tile_pool(name="outp", bufs=4))
        misc = ctx.enter_context(tc.tile_pool(name="misc", bufs=2))
        ppq = ctx.enter_context(tc.tile_pool(name="ppq", bufs=1, space="PSUM"))
        ppt = ctx.enter_context(tc.tile_pool(name="ppt", bufs=1, space="PSUM"))
        ppv = ctx.enter_context(tc.tile_pool(name="ppv", bufs=2, space="PSUM"))
        ppu = ctx.enter_context(tc.tile_pool(name="ppu", bufs=3, space="PSUM"))

        # ---- persistent constants ----
        # Both HWDGE queues (SP + ACT) stream x planes in parallel -- one
        # queue only pipelines ~2 transfers (~2us per 540KB plane), which
        # starves the qk conv (1.73us/plane consumption).  The small wqk
        # group loads are slotted around the ACT queue's planes so group g
        # always lands just before its taps run; wv/bqk/bvr follow (first
        # needed ~25us in).  DMAs are issued inside the b==0 plane loop.
        wqk_g = []
        for g in range(8):
            wt = const.tile([128, 4, 96], BF16, tag=f"wqk_g{g}")
            wqk_g.append(wt)
        bqk_sb = const.tile([96, 1], F32, tag="bqk_sb")
        wv_t = []
        for ck in range(8):
            wt = const.tile([128, 1024], BF16, tag=f"wv_sb{ck}")
            wv_t.append(wt)
        bvr_sb = const.tile([128, 1024], F32, tag="bvr_sb")
        # N=2 ones for the D-sum matmuls: fp32r requires an even innermost
        # moving count; f32r at N=2 is the cheapest legal form.  memset
        # can't write f32r, so round through an ACT copy.
        ones_f32 = const.tile([128, 2], F32, tag="ones_f32")
        nc.vector.memset(ones_f32[:], 1.0)
        ones2 = const.tile([128, 2], F32R, tag="ones2")
        nc.scalar.copy(ones2[:], ones_f32[:])
        # K/Q padded to 128 contraction rows: K=48 matmuls measure ~434 ns
        # (vs 215 at K=128) on this silicon, so zero-pad rows 48:128 once
        # and run the scores matmuls at full K.  f32r zeros == f32 zeros,
        # so memset through an F32 view is safe.
        Kp = const.tile([128, 1024], F32R, tag="Kp")
        Qp = const.tile([128, 1024], F32R, tag="Qp")
        nc.vector.memset(Kp[:].bitcast(F32), 0.0)
        nc.vector.memset(Qp[:].bitcast(F32), 0.0)

        for b in range(BPC):
            # ---- load parity-plane x, interleaved with the wqk groups in
            #      consumption order so the first matmul can start ~2us in ----
            xh = [[None] * 4 for _ in range(2)]
            if b == 0:
                first_wqk = True
                for h in range(2):
                    for pl in range(4):
                        k = h * 4 + pl
                        xt = xpool.tile([128, 2, 33, 32], BF16, tag="xh")
                        q = nc.sync if k % 2 == 0 else nc.scalar
                        if k % 2 == 1 and first_wqk:
                            # ACT queue: first 4 wqk groups ahead of planes
                            for g in range(4):
                                nc.scalar.dma_start(
                                    wqk_g[g][:], wqk.ap()[:, 4 * g : 4 * g + 4, :]
                                )
                            first_wqk = False
                        if k == 3:
                            for g in range(4, 8):
                                nc.scalar.dma_start(
                                    wqk_g[g][:], wqk.ap()[:, 4 * g : 4 * g + 4, :]
                                )
                        if k == 0:
                            # first plane split by shift so the very first
                            # taps (v=0) can start ~0.8us sooner
                            q.dma_start(xt[:, 0], xq.ap()[b, pl, ts(h, 128), 0])
                            q.dma_start(xt[:, 1], xq.ap()[b, pl, ts(h, 128), 1])
                        else:
                            q.dma_start(xt[:], xq.ap()[b, pl, ts(h, 128)])
                        xh[h][pl] = xt
                # rest of the ACT queue: bqk (needed ~25us), wv (~27us),
                # bvr (~30us)
                nc.scalar.dma_start(bqk_sb[:], bqk.ap())
                for ck in range(8):
                    nc.scalar.dma_start(wv_t[ck][:], wv.ap()[:, ck, :])
                nc.scalar.dma_start(bvr_sb[:], bvr.ap())
            else:
                # batch 1: SWDGE queue -- it is otherwise idle, and its
                # instruction stream reaches these issues immediately, so
                # the planes prefetch during batch 0's compute instead of
                # queueing behind batch 0's output DMAs on the HWDGE queues
                for k in range(8):
                    xt = xpool.tile([128, 2, 33, 32], BF16, tag="xh")
                    xh[k // 4][k % 4] = xt
                    nc.gpsimd.dma_start(xt[:], xq.ap()[b, k % 4, ts(k // 4, 128)])
            # v-conv stationary views: space-to-depth chunk (dy, dx) of
            # half h is rows u2..u2+32 of shifted plane (a2p, v2) -- a
            # contiguous [128, 1024] window straight into the plane tile.
            xs_c = []
            for ck in range(8):
                t, h = divmod(ck, 2)
                dy, dx = divmod(t, 2)
                a, u2 = (dy + 1) % 2, (dy + 1) // 2
                p2, v2 = (dx + 1) % 2, (dx + 1) // 2
                srcv = xh[h][a * 2 + p2][:, v2, u2 : u2 + 32, :]
                xs_c.append(srcv.rearrange("p a b -> p (a b)"))

            # ---- composite q|k conv: psum[96, 512] per m-chunk ----
            # plane-major tap order so matmuls start as soon as the first
            # plane's DMA lands.  K/Q stay fp32r: the scores matmul is the
            # one place bf16 noise measurably hurts (it feeds exp).  The
            # padded Kp/Qp then feed full-K scores matmuls.  Single pass
            # over the planes: both jm-half psums accumulate side by side,
            # so each plane is consumed once, right as its DMA lands.
            QK = qkp.tile([96, 1024], F32R, tag="QK")
            pq_t = []
            for jm in range(2):
                pqt = ppq.tile([96, 512], F32, tag=f"pq{jm}")
                pq_t.append(pqt)
            first = True
            for h in range(2):
                for pl in range(4):
                    for u in range(2):
                        for v in range(2):
                            ck2 = h * 16 + pl * 4 + u * 2 + v
                            last = h == 1 and pl == 3 and u == 1 and v == 1
                            for jm in range(2):
                                rhs = xh[h][pl][
                                    :, v, u + 16 * jm : u + 16 * jm + 16, :
                                ]
                                nc.tensor.matmul(
                                    pq_t[jm][:], wqk_g[ck2 // 4][:, ck2 % 4, :],
                                    rhs, start=first, stop=last,
                                )
                            first = False
            for jm in range(2):
                nc.vector.tensor_scalar_add(
                    QK[:, ts(jm, 512)], pq_t[jm][:], bqk_sb[:, :1]
                )
                # k rows land partition-aligned -> DVE copy into padded Kp
                nc.vector.tensor_copy(Kp[0:48, ts(jm, 512)], QK[0:48, ts(jm, 512)])
            # q rows (48:96) need a partition move -> DMA into padded Qp
            # (SP HWDGE: SWDGE dies on SBUF->SBUF moves)
            nc.sync.dma_start(Qp[0:48, :], QK[48:96, :])

            # ---- v conv (V^T tiles) with the transposed-scores matmuls,
            #      exps, and Esum partial sums interleaved so the in-order
            #      PE never sits behind an ACT exp or a psum-bank release ----
            e_sb = epool.tile([128, 8, 1024], BF16, tag="e_sb")
            vt_sb = vtpool.tile([128, 8, 1024], BF16, tag="vt_sb")
            esum = epool.tile([128, 1024], F32R, tag="esum")
            def scores_chunk(sc):
                # scores chunk: T[n, m] = S[m, n] for n-chunk sc//2,
                # m-half sc%2 -- a single matmul + exp slotted between the
                # v-conv groups.  Esum partials build as chunks complete
                # (D[m] then needs only one 128-deep matmul per m-chunk).
                sn, sm = divmod(sc, 2)
                pt_t = ppt.tile([128, 512], F32, tag="pt")
                nc.tensor.matmul(
                    pt_t[:], Kp[:, ts(sn, 128)], Qp[:, ts(sm, 512)],
                    start=True, stop=True,
                )
                nc.scalar.activation(e_sb[:, sn, ts(sm, 512)], pt_t[:], EXP)
                if sc % 2 == 1:
                    if sn == 1:
                        nc.any.tensor_add(esum[:], e_sb[:, 0, :], e_sb[:, 1, :])
                    elif sn > 1:
                        nc.any.tensor_add(esum[:], esum[:], e_sb[:, sn, :])

            for g in range(16):
                jn, l = divmod(g, 2)
                pv_t = ppv.tile([128, 512], F32, tag="pv")
                for ck in range(8):
                    nc.tensor.matmul(
                        pv_t[:], xs_c[ck][:, ts(jn, 128)], wv_t[ck][:, ts(l, 512)],
                        start=(ck == 0), stop=(ck == 7),
                    )
                nc.vector.tensor_add(
                    vt_sb[:, jn, ts(l, 512)], pv_t[:], bvr_sb[:, ts(l, 512)]
                )
                # scores lag the v groups by 2 so the first chunk never
                # waits on the Qp partition-move DMA
                if g >= 2:
                    scores_chunk(g - 2)
            scores_chunk(14)
            scores_chunk(15)

            # ---- D[m] + reciprocals for ALL m-chunks up front, so the U
            #      loop's evictions never wait on the rc dependency chain ----
            rcs = []
            for mm in range(8):
                # D psums borrow the scores bank (scores are done by now)
                pd_t = ppt.tile([128, 512], F32, tag="pt")
                nc.tensor.matmul(
                    pd_t[:, 0:2], esum[:, ts(mm, 128)], ones2[:], start=True, stop=True
                )
                rc = misc.tile([128, 1], F32, tag=f"rc{mm}")
                nc.vector.reciprocal(rc[:], pd_t[:, 0:1])
                rcs.append(rc)

            # ---- U^T[m, c] = sum_n E[n, m] V^T[n, c]; out^T = U^T/D ----
            for mm in range(8):
                for l in range(2):
                    pu_t = ppu.tile([128, 512], F32, tag="pu")
                    for jn in range(8):
                        nc.tensor.matmul(
                            pu_t[:], e_sb[:, jn, ts(mm, 128)], vt_sb[:, jn, ts(l, 512)],
                            start=(jn == 0), stop=(jn == 7),
                        )
                    ot = outp.tile([128, 512], BF16, tag="ot")
                    nc.scalar.activation(ot[:], pu_t[:], COPY, scale=rcs[mm][:])
                    # alternate the two HWDGE queues at the tail
                    q = nc.sync if l == 0 else nc.scalar
                    q.dma_start(o.ap()[b, mm, :, ts(l, 512)], ot[:])

    nc.compile()
    return nc


def host_weights(dc_w, dc_b, q_w, k_w, q_b, k_b, v_w, v_b):
    """Fold dc conv into q/k projections -> composite 4x4 stride-2 weights."""
    dc_w = np.asarray(dc_w, np.float32)
    dc_b = np.asarray(dc_b, np.float32)
    q_w = np.asarray(q_w, np.float32)
    k_w = np.asarray(k_w, np.float32)
    q_b = np.asarray(q_b, np.float32)
    k_b = np.asarray(k_b, np.float32)
    v_w = np.asarray(v_w, np.float32)
    v_b = np.asarray(v_b, np.float32)

    C = dc_w.shape[1]
    Wq = np.zeros((48, C, 4, 4), np.float64)
    Wk = np.zeros((48, C, 4, 4), np.float64)
    for p in range(2):
        for qq in range(2):
            qw_pq = q_w[:, :, p, qq].astype(np.float64)
            kw_pq = k_w[:, :, p, qq].astype(np.float64)
            for dy in range(3):
                for dx in range(3):
                    dcw_dd = dc_w[:, :, dy, dx].astype(np.float64)
                    Wq[:, :, p + dy, qq + dx] += qw_pq @ dcw_dd
                    Wk[:, :, p + dy, qq + dx] += kw_pq @ dcw_dd
    bq_eff = q_b + q_w.sum(axis=(2, 3)) @ dc_b
    bk_eff = k_b + k_w.sum(axis=(2, 3)) @ dc_b
    # lhsT row index = (A*4+B)*C + c', columns: k 0:48 | q 48:96
    # (the device uses QK[0:48] as the scores lhsT (indexes n -> k) and
    #  QK[48:96] as the scores rhs (indexes m -> q))
    wqk_ab = (
        np.concatenate(
            [
                Wk.transpose(2, 3, 1, 0).reshape(16 * C, 48),
                Wq.transpose(2, 3, 1, 0).reshape(16 * C, 48),
            ],
            axis=1,
        )
        .astype(np.float32)
        .reshape(32, 128, 96)  # chunk_old = (A*4+B)*2 + h
    )
    # permute chunks into device consumption order (h, pl, u, v)
    perm = []
    for h in range(2):
        for pl in range(4):
            a, p = divmod(pl, 2)
            for u in range(2):
                for v in range(2):
                    A, Bo = 2 * u + a, 2 * v + p
                    perm.append((A * 4 + Bo) * 2 + h)
    wqk = np.ascontiguousarray(
        wqk_ab[perm].transpose(1, 0, 2).astype(NP_BF16)
    )  # [part 128, chunk2 32, 96]
    bqk = np.concatenate([bk_eff, bq_eff]).reshape(96, 1).astype(np.float32)
    # v rhs: row = (dy*2+dx)*C + c', col = oc
    wv = np.ascontiguousarray(
        v_w.transpose(2, 3, 1, 0).reshape(8, 128, 4 * C).transpose(1, 0, 2)
        .astype(NP_BF16)
    )  # [part 128, chunk 8, oc]
    bvr = np.ascontiguousarray(np.broadcast_to(v_b, (128, 4 * C))).astype(np.float32)
    return wqk, bqk, wv, bvr


_PROGRAM = None
LAST_RESULTS = None


def _get_program():
    global _PROGRAM
    if _PROGRAM is None:
        _PROGRAM = build_program()
    return _PROGRAM


def kernel(x, dc_w, dc_b, q_w, q_b, k_w, k_b, v_w, v_b):
    _install_ntff_hook_shim()
    x = np.asarray(x, np.float32)
    B = x.shape[0]
    xp = np.pad(x, ((0, 0), (0, 0), (1, 1), (1, 1)))
    # parity planes: pq[b, c, a*2+p, r, s] = x_pad[b, c, 2r+a, 2s+p],
    # 2 column shifts, then plane-major: xq[b, pl, c, sv, r, j]
    pq = (
        xp.reshape(B, C_IN, 33, 2, 33, 2)
        .transpose(0, 1, 3, 5, 2, 4)
        .reshape(B, C_IN, 4, 33, 33)
        .astype(NP_BF16)
    )
    xq = np.stack([pq[..., 0:32], pq[..., 1:33]], axis=3).transpose(0, 2, 1, 3, 4, 5)
    wqk, bqk, wv, bvr = host_weights(dc_w, dc_b, q_w, k_w, q_b, k_b, v_w, v_b)

    nc = _get_program()
    in_maps = []
    for c in range(NCORES):
        in_maps.append(
            {
                "xq": np.ascontiguousarray(xq[BPC * c : BPC * (c + 1)]),
                "wqk": wqk,
                "wv": wv,
                "bqk": bqk,
                "bvr": bvr,
            }
        )
    res = bass_utils.run_bass_kernel_spmd(nc, in_maps, core_ids=list(range(NCORES)))
    global LAST_RESULTS
    LAST_RESULTS = res

    out = np.empty((B, 1024, 1024), np.float32)
    for c in range(NCORES):
        # o[b, mm, r, j] = out^T[mm*128+r, j] -> out[b, oc, m]
        oc_ = np.asarray(res.results[c]["o"]).astype(np.float32)
        ot = oc_.reshape(BPC, 1024, 1024)
        out[BPC * c : BPC * (c + 1)] = ot.transpose(0, 2, 1)
    return out


# revision 40
# speedup vs baseline: 1.0575x; 1.0575x over previous
"""Trainium2 Bass kernel for nn_DGM_77318001263213 (dense_transformer).

Reference computation (per batch b of 16):
  dir_map = conv3x3_SAME(x, dc_w) + dc_b            [12, 64, 64]
  q = conv2x2_s2(dir_map, q_w) + q_b  -> [48, 1024]
  k = conv2x2_s2(dir_map, k_w) + k_b  -> [48, 1024]
  v = conv2x2_s2(x, v_w) + v_b        -> [1024, 1024]
  attn = softmax(q^T k, axis=-1)                    [1024, 1024]
  out[c, m] = sum_n v[c, n] * attn[m, n]            [1024, 1024]

Device mapping (data-parallel, 2 batches per core on 8 cores):
  * q,k are computed as ONE composite 4x4 stride-2 convolution of x
    (the 3x3 dc conv and 2x2 proj convs are both linear, so they fold
    into a single 4x4 kernel on the host) with 96 output channels
    (q stacked with k) -- keeps PE matmul M=96 instead of M=12.
  * v conv computed transposed: V^T[n, oc] tiles via matmuls with
    lhsT = space-to-depth x (derived on device from the parity planes),
    rhs = reshaped v weights.
  * scores computed transposed: T[n, m] = S[m, n] = sum_c k[c,n] q[c,m],
    then E = exp(T) (no max subtraction; |S| <= ~25 so exp is safe),
    unnormalized U^T[m, c] = sum_n E[n, m] V^T[n, c], row sums
    D[m] = sum_n E[n, m] via ones-matmul, and out^T[m, c] =
    U^T[m, c] / D[m] applied as a per-partition scale on eviction.
  * matmul operands are bf16 (x, weights, E, V^T) except the scores
    matmul (QK/Qs stay fp32r: q/k noise feeds exp and is the one place
    extra mantissa bits pay for themselves).  PE rate is identical for
    bf16/fp32r (~250 ns per 512-col matmul, ~2.05 GHz sustained; fp8
    DoubleRow measured only 2x per instruction, which precision-mandated
    hi/lo splits would exactly cancel -- so no fp8).  bf16 halves DMA
    and SBUF instead.
  * host transposes out^T -> out at gather time.
"""
import os
import sys
import types
import numpy as np
from contextlib import ExitStack

for _p in ("/opt/trn_rl_repo", "/root/.axon_site/_ro/trn_rl_repo"):
    if os.path.isdir(_p) and _p not in sys.path:
        sys.path.insert(0, _p)

import ml_dtypes
import concourse.bacc as bacc
import concourse.bass as bass
import concourse.tile as tile
import concourse.mybir as mybir
from concourse import bass_utils

F32 = mybir.dt.float32
F32R = mybir.dt.float32r
BF16 = mybir.dt.bfloat16
NP_BF16 = ml_dtypes.bfloat16
ts = bass.ts

NCORES = 8
BPC = 2          # batches per core
C_IN = 256
NPOS = 1024      # 32*32 output positions
OC = 1024        # v output channels


def _install_ntff_hook_shim():
    """Register the axon NTFF profile hook if the image's antenv lacks it.

    Only needed when BASS_TRACE=1; harmless otherwise."""
    if "antenv.axon_hooks" in sys.modules:
        return
    try:
        from trn_agent_boot.trn_boot import _ntff_profile_via_ctypes
        hook = _ntff_profile_via_ctypes("/opt/axon/libaxon_pjrt.so")
    except Exception:
        hook = None
    m = types.ModuleType("antenv.axon_hooks")
    m.get_axon_ntff_profile_hook = lambda: hook
    m.set_axon_ntff_profile_hook = lambda h: None
    sys.modules["antenv.axon_hooks"] = m


def build_program():
    """Build the per-core Bacc program (same program on all 8 cores)."""
    nc = bacc.Bacc(trn_type="TRN2", target_bir_lowering=False, debug=False)

    # padded x as 4 stride-2 parity planes, each duplicated at 2 column
    # shifts: xq[b, c, a*2+p, sv, r, j] = x_pad[b, c, 2r+a, 2(sv+j)+p].
    # The shift copies make every conv tap view a fully CONTIGUOUS
    # [128, 512] / [128, 1024] run (32-wide 64B-aligned rows), which the
    # PE ifmap fetcher streams at full rate (strided 33-wide views
    # measured ~263 ns vs 225 ns per matmul), and lets the v conv use
    # the planes directly as its stationary operand (no space-to-depth
    # copies on device at all).  Costs 2x the x DMA bytes -- DMA has
    # ~3x headroom over the PE here.
    # plane-major so each (half, plane) DMA reads one contiguous 540KB
    # region (strided per-channel reads measured ~150 GB/s vs wire 358)
    xq = nc.dram_tensor("xq", [BPC, 4, C_IN, 2, 33, 32], BF16, kind="ExternalInput")
    # wqk chunk order = (h, pl, u, v) == the exact consumption order of the
    # q/k conv loop, so the per-group DMAs stream just ahead of the matmuls
    wqk = nc.dram_tensor("wqk", [128, 32, 96], BF16, kind="ExternalInput")
    wv = nc.dram_tensor("wv", [128, 8, 1024], BF16, kind="ExternalInput")
    bqk = nc.dram_tensor("bqk", [96, 1], F32, kind="ExternalInput")
    bvr = nc.dram_tensor("bvr", [128, 1024], F32, kind="ExternalInput")
    # bf16 + tile-major output: each [128, 512] eviction writes one
    # contiguous 131KB region (strided writes measured ~37 GB/s -- they
    # were the entire end-of-program drain); host reassembles + converts
    o = nc.dram_tensor("o", [BPC, 8, 2, 128, 512], BF16, kind="ExternalOutput")

    EXP = mybir.ActivationFunctionType.Exp
    COPY = mybir.ActivationFunctionType.Copy

    with tile.TileContext(nc) as tc, ExitStack() as ctx:
        const = ctx.enter_context(tc.tile_pool(name="const", bufs=1))
        xpool = ctx.enter_context(tc.tile_pool(name="xpool", bufs=16))
        qkp = ctx.enter_context(tc.tile_pool(name="qkp", bufs=1))
        epool = ctx.enter_context(tc.tile_pool(name="epool", bufs=1))
        vtpool = ctx.enter_context(tc.tile_pool(name="vtpool", bufs=1))
        outp = ctx.enter_context(tc.tile_pool(name="outp", bufs=4))
        misc = ctx.enter_context(tc.tile_pool(name="misc", bufs=2))
        ppq = ctx.enter_context(tc.tile_pool(name="ppq", bufs=1, space="PSUM"))
        ppt = ctx.enter_context(tc.tile_pool(name="ppt", bufs=1, space="PSUM"))
        ppv = ctx.enter_context(tc.tile_pool(name="ppv", bufs=2, space="PSUM"))
        ppu = ctx.enter_context(tc.tile_pool(name="ppu", bufs=3, space="PSUM"))

        # ---- persistent constants ----
        # Both HWDGE queues (SP + ACT) stream x planes in parallel -- one
        # queue only pipelines ~2 transfers (~2us per 540KB plane), which
        # starves the qk conv (1.73us/plane consumption).  The small wqk
        # group loads are slotted around the ACT queue's planes so group g
        # always lands just before its taps run; wv/bqk/bvr follow (first
        # needed ~25us in).  DMAs are issued inside the b==0 plane loop.
        wqk_g = []
        for g in range(8):
            wt = const.tile([128, 4, 96], BF16, tag=f"wqk_g{g}")
            wqk_g.append(wt)
        bqk_sb = const.tile([96, 1], F32, tag="bqk_sb")
        wv_t = []
        for ck in range(8):
            wt = const.tile([128, 1024], BF16, tag=f"wv_sb{ck}")
            wv_t.append(wt)
        bvr_sb = const.tile([128, 1024], F32, tag="bvr_sb")
        # N=2 ones for the D-sum matmuls: fp32r requires an even innermost
        # moving count; f32r at N=2 is the cheapest legal form.  memset
        # can't write f32r, so round through an ACT copy.
        ones_f32 = const.tile([128, 2], F32, tag="ones_f32")
        nc.vector.memset(ones_f32[:], 1.0)
        ones2 = const.tile([128, 2], F32R, tag="ones2")
        nc.scalar.copy(ones2[:], ones_f32[:])
        # K/Q padded to 128 contraction rows: K=48 matmuls measure ~434 ns
        # (vs 215 at K=128) on this silicon, so zero-pad rows 48:128 once
        # and run the scores matmuls at full K.  f32r zeros == f32 zeros,
        # so memset through an F32 view is safe.
        Kp = const.tile([128, 1024], F32R, tag="Kp")
        Qp = const.tile([128, 1024], F32R, tag="Qp")
        nc.vector.memset(Kp[:].bitcast(F32), 0.0)
        nc.vector.memset(Qp[:].bitcast(F32), 0.0)

        for b in range(BPC):
            # ---- load parity-plane x, interleaved with the wqk groups in
            #      consumption order so the first matmul can start ~2us in ----
            xh = [[None] * 4 for _ in range(2)]
            if b == 0:
                first_wqk = True
                for h in range(2):
                    for pl in range(4):
                        k = h * 4 + pl
                        xt = xpool.tile([128, 2, 33, 32], BF16, tag="xh")
                        q = nc.sync if k % 2 == 0 else nc.scalar
                        if k % 2 == 1 and first_wqk:
                            # ACT queue: first 4 wqk groups ahead of planes
                            for g in range(4):
                                nc.scalar.dma_start(
                                    wqk_g[g][:], wqk.ap()[:, 4 * g : 4 * g + 4, :]
                                )
                            first_wqk = False
                        if k == 3:
                            for g in range(4, 8):
                                nc.scalar.dma_start(
                                    wqk_g[g][:], wqk.ap()[:, 4 * g : 4 * g + 4, :]
                                )
                        if k == 0:
                            # first plane split by shift so the very first
                            # taps (v=0) can start ~0.8us sooner
                            q.dma_start(xt[:, 0], xq.ap()[b, pl, ts(h, 128), 0])
                            q.dma_start(xt[:, 1], xq.ap()[b, pl, ts(h, 128), 1])
                        else:
                            q.dma_start(xt[:], xq.ap()[b, pl, ts(h, 128)])
                        xh[h][pl] = xt
                # rest of the ACT queue: bqk (needed ~25us), wv (~27us),
                # bvr (~30us)
                nc.scalar.dma_start(bqk_sb[:], bqk.ap())
                for ck in range(8):
                    nc.scalar.dma_start(wv_t[ck][:], wv.ap()[:, ck, :])
                nc.scalar.dma_start(bvr_sb[:], bvr.ap())
            else:
                # batch 1: alternate the two HWDGE queues like batch 0
                for k in range(8):
                    xt = xpool.tile([128, 2, 33, 32], BF16, tag="xh")
                    xh[k // 4][k % 4] = xt
                    q = nc.sync if k % 2 == 0 else nc.scalar
                    q.dma_start(xt[:], xq.ap()[b, k % 4, ts(k // 4, 128)])
            # v-conv stationary views: space-to-depth chunk (dy, dx) of
            # half h is rows u2..u2+32 of shifted plane (a2p, v2) -- a
            # contiguous [128, 1024] window straight into the plane tile.
            xs_c = []
            for ck in range(8):
                t, h = divmod(ck, 2)
                dy, dx = divmod(t, 2)
                a, u2 = (dy + 1) % 2, (dy + 1) // 2
                p2, v2 = (dx + 1) % 2, (dx + 1) // 2
                srcv = xh[h][a * 2 + p2][:, v2, u2 : u2 + 32, :]
                xs_c.append(srcv.rearrange("p a b -> p (a b)"))

            # ---- composite q|k conv: psum[96, 512] per m-chunk ----
            # plane-major tap order so matmuls start as soon as the first
            # plane's DMA lands.  K/Q stay fp32r: the scores matmul is the
            # one place bf16 noise measurably hurts (it feeds exp).  The
            # padded Kp/Qp then feed full-K scores matmuls.  Single pass
            # over the planes: both jm-half psums accumulate side by side,
            # so each plane is consumed once, right as its DMA lands.
            QK = qkp.tile([96, 1024], F32R, tag="QK")
            pq_t = []
            for jm in range(2):
                pqt = ppq.tile([96, 512], F32, tag=f"pq{jm}")
                pq_t.append(pqt)
            first = True
            for h in range(2):
                for pl in range(4):
                    for u in range(2):
                        for v in range(2):
                            ck2 = h * 16 + pl * 4 + u * 2 + v
                            last = h == 1 and pl == 3 and u == 1 and v == 1
                            for jm in range(2):
                                rhs = xh[h][pl][
                                    :, v, u + 16 * jm : u + 16 * jm + 16, :
                                ]
                                nc.tensor.matmul(
                                    pq_t[jm][:], wqk_g[ck2 // 4][:, ck2 % 4, :],
                                    rhs, start=first, stop=last,
                                )
                            first = False
            for jm in range(2):
                nc.vector.tensor_scalar_add(
                    QK[:, ts(jm, 512)], pq_t[jm][:], bqk_sb[:, :1]
                )
                # k rows land partition-aligned -> DVE copy into padded Kp
                nc.vector.tensor_copy(Kp[0:48, ts(jm, 512)], QK[0:48, ts(jm, 512)])
            # q rows (48:96) need a partition move -> DMA into padded Qp
            # (SP HWDGE: SWDGE dies on SBUF->SBUF moves)
            nc.sync.dma_start(Qp[0:48, :], QK[48:96, :])

            # ---- v conv (V^T tiles) with the transposed-scores matmuls,
            #      exps, and Esum partial sums interleaved so the in-order
            #      PE never sits behind an ACT exp or a psum-bank release ----
            e_sb = epool.tile([128, 8, 1024], BF16, tag="e_sb")
            vt_sb = vtpool.tile([128, 8, 1024], BF16, tag="vt_sb")
            esum = epool.tile([128, 1024], F32R, tag="esum")
            def scores_chunk(sc):
                # scores chunk: T[n, m] = S[m, n] for n-chunk sc//2,
                # m-half sc%2 -- a single matmul + exp slotted between the
                # v-conv groups.  Esum partials build as chunks complete
                # (D[m] then needs only one 128-deep matmul per m-chunk).
                sn, sm = divmod(sc, 2)
                pt_t = ppt.tile([128, 512], F32, tag="pt")
                nc.tensor.matmul(
                    pt_t[:], Kp[:, ts(sn, 128)], Qp[:, ts(sm, 512)],
                    start=True, stop=True,
                )
                nc.scalar.activation(e_sb[:, sn, ts(sm, 512)], pt_t[:], EXP)
                if sc % 2 == 1:
                    if sn == 1:
                        nc.any.tensor_add(esum[:], e_sb[:, 0, :], e_sb[:, 1, :])
                    elif sn > 1:
                        nc.any.tensor_add(esum[:], esum[:], e_sb[:, sn, :])

            for g in range(16):
                jn, l = divmod(g, 2)
                pv_t = ppv.tile([128, 512], F32, tag="pv")
                for ck in range(8):
                    nc.tensor.matmul(
                        pv_t[:], xs_c[ck][:, ts(jn, 128)], wv_t[ck][:, ts(l, 512)],
                        start=(ck == 0), stop=(ck == 7),
                    )
                nc.vector.tensor_add(
                    vt_sb[:, jn, ts(l, 512)], pv_t[:], bvr_sb[:, ts(l, 512)]
                )
                # scores lag the v groups by 2 so the first chunk never
                # waits on the Qp partition-move DMA
                if g >= 2:
                    scores_chunk(g - 2)
            scores_chunk(14)
            scores_chunk(15)

            # ---- D[m] + reciprocals for ALL m-chunks up front, so the U
            #      loop's evictions never wait on the rc dependency chain ----
            rcs = []
            for mm in range(8):
                # D psums borrow the scores bank (scores are done by now)
                pd_t = ppt.tile([128, 512], F32, tag="pt")
                nc.tensor.matmul(
                    pd_t[:, 0:2], esum[:, ts(mm, 128)], ones2[:], start=True, stop=True
                )
                rc = misc.tile([128, 1], F32, tag=f"rc{mm}")
                nc.vector.reciprocal(rc[:], pd_t[:, 0:1])
                rcs.append(rc)

            # ---- U^T[m, c] = sum_n E[n, m] V^T[n, c]; out^T = U^T/D ----
            for mm in range(8):
                for l in range(2):
                    pu_t = ppu.tile([128, 512], F32, tag="pu")
                    for jn in range(8):
                        nc.tensor.matmul(
                            pu_t[:], e_sb[:, jn, ts(mm, 128)], vt_sb[:, jn, ts(l, 512)],
                            start=(jn == 0), stop=(jn == 7),
                        )
                    ot = outp.tile([128, 512], BF16, tag="ot")
                    nc.scalar.activation(ot[:], pu_t[:], COPY, scale=rcs[mm][:])
                    # alternate the two HWDGE queues at the tail; each
                    # half-tile write is one contiguous 131KB region
                    q = nc.sync if l == 0 else nc.scalar
                    q.dma_start(o.ap()[b, mm, l], ot[:])

    nc.compile()
    return nc


def host_weights(dc_w, dc_b, q_w, k_w, q_b, k_b, v_w, v_b):
    """Fold dc conv into q/k projections -> composite 4x4 stride-2 weights."""
    dc_w = np.asarray(dc_w, np.float32)
    dc_b = np.asarray(dc_b, np.float32)
    q_w = np.asarray(q_w, np.float32)
    k_w = np.asarray(k_w, np.float32)
    q_b = np.asarray(q_b, np.float32)
    k_b = np.asarray(k_b, np.float32)
    v_w = np.asarray(v_w, np.float32)
    v_b = np.asarray(v_b, np.float32)

    C = dc_w.shape[1]
    Wq = np.zeros((48, C, 4, 4), np.float64)
    Wk = np.zeros((48, C, 4, 4), np.float64)
    for p in range(2):
        for qq in range(2):
            qw_pq = q_w[:, :, p, qq].astype(np.float64)
            kw_pq = k_w[:, :, p, qq].astype(np.float64)
            for dy in range(3):
                for dx in range(3):
                    dcw_dd = dc_w[:, :, dy, dx].astype(np.float64)
                    Wq[:, :, p + dy, qq + dx] += qw_pq @ dcw_dd
                    Wk[:, :, p + dy, qq + dx] += kw_pq @ dcw_dd
    bq_eff = q_b + q_w.sum(axis=(2, 3)) @ dc_b
    bk_eff = k_b + k_w.sum(axis=(2, 3)) @ dc_b
    # lhsT row index = (A*4+B)*C + c', columns: k 0:48 | q 48:96
    # (the device uses QK[0:48] as the scores lhsT (indexes n -> k) and
    #  QK[48:96] as the scores rhs (indexes m -> q))
    wqk_ab = (
        np.concatenate(
            [
                Wk.transpose(2, 3, 1, 0).reshape(16 * C, 48),
                Wq.transpose(2, 3, 1, 0).reshape(16 * C, 48),
            ],
            axis=1,
        )
        .astype(np.float32)
        .reshape(32, 128, 96)  # chunk_old = (A*4+B)*2 + h
    )
    # permute chunks into device consumption order (h, pl, u, v)
    perm = []
    for h in range(2):
        for pl in range(4):
            a, p = divmod(pl, 2)
            for u in range(2):
                for v in range(2):
                    A, Bo = 2 * u + a, 2 * v + p
                    perm.append((A * 4 + Bo) * 2 + h)
    wqk = np.ascontiguousarray(
        wqk_ab[perm].transpose(1, 0, 2).astype(NP_BF16)
    )  # [part 128, chunk2 32, 96]
    bqk = np.concatenate([bk_eff, bq_eff]).reshape(96, 1).astype(np.float32)
    # v rhs: row = (dy*2+dx)*C + c', col = oc
    wv = np.ascontiguousarray(
        v_w.transpose(2, 3, 1, 0).reshape(8, 128, 4 * C).transpose(1, 0, 2)
        .astype(NP_BF16)
    )  # [part 128, chunk 8, oc]
    bvr = np.ascontiguousarray(np.broadcast_to(v_b, (128, 4 * C))).astype(np.float32)
    return wqk, bqk, wv, bvr


_PROGRAM = None
LAST_RESULTS = None


def _get_program():
    global _PROGRAM
    if _PROGRAM is None:
        _PROGRAM = build_program()
    return _PROGRAM


def kernel(x, dc_w, dc_b, q_w, q_b, k_w, k_b, v_w, v_b):
    _install_ntff_hook_shim()
    x = np.asarray(x, np.float32)
    B = x.shape[0]
    xp = np.pad(x, ((0, 0), (0, 0), (1, 1), (1, 1)))
    # parity planes: pq[b, c, a*2+p, r, s] = x_pad[b, c, 2r+a, 2s+p],
    # 2 column shifts, then plane-major: xq[b, pl, c, sv, r, j]
    pq = (
        xp.reshape(B, C_IN, 33, 2, 33, 2)
        .transpose(0, 1, 3, 5, 2, 4)
        .reshape(B, C_IN, 4, 33, 33)
        .astype(NP_BF16)
    )
    xq = np.stack([pq[..., 0:32], pq[..., 1:33]], axis=3).transpose(0, 2, 1, 3, 4, 5)
    wqk, bqk, wv, bvr = host_weights(dc_w, dc_b, q_w, k_w, q_b, k_b, v_w, v_b)

    nc = _get_program()
    in_maps = []
    for c in range(NCORES):
        in_maps.append(
            {
                "xq": np.ascontiguousarray(xq[BPC * c : BPC * (c + 1)]),
                "wqk": wqk,
                "wv": wv,
                "bqk": bqk,
                "bvr": bvr,
            }
        )
    res = bass_utils.run_bass_kernel_spmd(nc, in_maps, core_ids=list(range(NCORES)))
    global LAST_RESULTS
    LAST_RESULTS = res

    out = np.empty((B, 1024, 1024), np.float32)
    for c in range(NCORES):
        # o[b, mm, l, r, j] = out^T[mm*128+r, l*512+j] -> out[b, oc, m]
        oc_ = np.asarray(res.results[c]["o"]).astype(np.float32)
        ot = oc_.transpose(0, 1, 3, 2, 4).reshape(BPC, 1024, 1024)
        out[BPC * c : BPC * (c + 1)] = ot.transpose(0, 2, 1)
    return out
